# revision 24
# baseline (speedup 1.0000x reference)
"""DenseGGNN (gnn_message_passing) Trainium2 Bass kernel.

Math per layer i (per batch):
    s  = A^T @ h                    # [N, C], A binary adjacency
    gx = s @ (W_i @ w_ih_i^T)       # fused:  ((A^T h) W) @ w_ih^T
    gh = h @ w_hh_i^T
    r  = sigmoid(gx_r + gh_r + b_r);  zc = 1 - z = sigmoid(-(gx_z + gh_z + b_z))
    n  = tanh(gx_n + b_in + r * (gh_n + b_hn))
    h' = h + zc * (n - h)

The network amplifies early-layer errors ~40-170x into the output, so
precision tapers by layer (budgets measured by emulation against the
fp32 reference):
  - s-operand: layer 0 runs two fp16 passes over a host-supplied
    hi/lo split of x (~21 bits; fp16 single here costs 1.7e-2 final);
    layers 1-3 use the single-fp16 node-major state.
  - fused gate weight wc: hi+lo fp16 split (wch + wcl) applied in
    layers 0-2 (fp16-single wc costs 1.5e-2 at layer 1); layer 3 single.
  - s is cast to a hi/lo fp16 pair in layer 0, single fp16 after.
  - state/gh/everything else: fp16 (insensitive).
Expected final rel err ~3e-3 vs the 2e-2 gate.

The adjacency is fp8e4 (exact for 0/1 - halves HBM traffic vs fp16);
the PE consumes it as a mixed fp16xfp8 matmul.  Layout: state hT is
feature-major fp16 [C=128 part, N]; the s-matmul lhsT is node-major,
regenerated between layers by per-half DMA xbar transposes.  The
output is stored feature-major fp16 and transposed/widened on the
host.  PSUM accumulates fp32.

Sharding: batch (32) split across 8 cores, 4 batches/core, weights
replicated; no cross-core communication.
"""

from contextlib import ExitStack, nullcontext

import numpy as np

import concourse.bass as bass
import concourse.bacc as bacc
import concourse.tile as tile
import concourse.mybir as mybir
from concourse.bass_utils import run_bass_kernel_spmd

B, N, C, L = 32, 1024, 128, 4
NCORES = 8
BPC = B // NCORES          # batches per core
P = 128                    # partitions
NT = N // P                # node tiles (8)
HALF = 512                 # psum-bank-sized column chunk

F32 = mybir.dt.float32
F16 = mybir.dt.float16
F8 = mybir.dt.float8e4
AF = mybir.ActivationFunctionType
ALU = mybir.AluOpType

WC_SPLIT = (True, True, True, False)   # layers applying the wcl correction

_PROGRAM_CACHE = {}


def _build_program(reps: int = 1, loop_reps: int = 1) -> bass.Bass:
    # reps > 1 re-emits the whole body back-to-back in one NEFF;
    # loop_reps > 1 wraps the body in a hardware For_i loop.  Both are
    # benchmarking aids (wall-time slope isolates per-iteration device
    # time from the axon dispatch overhead).
    nc = bacc.Bacc()

    xh_d = nc.declare_dram_parameter("xh", [BPC, N, C], F16, isOutput=False)
    xl_d = nc.declare_dram_parameter("xl", [BPC, N, C], F16, isOutput=False)
    xt_d = nc.declare_dram_parameter("xt", [BPC, C, N], F16, isOutput=False)
    adj_d = nc.declare_dram_parameter("adj", [BPC, N, N], F8, isOutput=False)
    wch_d = nc.declare_dram_parameter("wch", [C, L, 3, C], F16, isOutput=False)
    wcl_d = nc.declare_dram_parameter("wcl", [C, L, 3, C], F16, isOutput=False)
    whh_d = nc.declare_dram_parameter("whh", [C, L, 3, C], F16, isOutput=False)
    bias_d = nc.declare_dram_parameter("bias", [C, L, 4], F32, isOutput=False)
    y_d = nc.declare_dram_parameter("y", [BPC, C, N], F16, isOutput=True)

    with tile.TileContext(nc) as tc, ExitStack() as ctx:
        consts = ctx.enter_context(tc.tile_pool(name="consts", bufs=1))
        adj_pool = ctx.enter_context(tc.tile_pool(name="adjp", bufs=1))
        hnm_pool = ctx.enter_context(tc.tile_pool(name="hnm", bufs=2))
        xlo_pool = ctx.enter_context(tc.tile_pool(name="xlo", bufs=1))
        hT_pool = ctx.enter_context(tc.tile_pool(name="hT", bufs=2))
        sT_pool = ctx.enter_context(tc.tile_pool(name="sT", bufs=2))
        ew_pool = ctx.enter_context(tc.tile_pool(name="ew", bufs=10))
        ps_s = ctx.enter_context(tc.tile_pool(name="ps_s", bufs=2, space="PSUM"))
        ps_g = ctx.enter_context(tc.tile_pool(name="ps_g", bufs=6, space="PSUM"))

        def wslice(w, i, g):
            return w[:, (i * 3 + g) * C:(i * 3 + g + 1) * C]

        def bslice(i, k):
            return bias[:, i * 4 + k:i * 4 + k + 1]

        loop_cm = (tc.For_i(0, loop_reps, 1, hint_engines=(mybir.EngineType.PE,))
                   if loop_reps > 1 else nullcontext())
        with loop_cm:
          for _rep in range(reps):
            # ---- input loads -------------------------------------------------
            adj_sb = [None] * BPC
            h_nm = [None] * BPC
            x_lo = [None] * BPC
            hT = [None] * BPC

            def load_x(b, eng):
                hi = hnm_pool.tile([P, NT, C], F16, tag=f"hnm{b}", name="hi")
                eng.dma_start(hi[:], xh_d[b].rearrange("(t p) c -> p t c", p=P))
                h_nm[b] = hi
                lo = xlo_pool.tile([P, NT, C], F16, tag=f"xlo{b}", name="lo")
                eng.dma_start(lo[:], xl_d[b].rearrange("(t p) c -> p t c", p=P))
                x_lo[b] = lo

            def load_ht(b, eng):
                h = hT_pool.tile([P, N], F16, tag=f"hT{b}", name="h")
                eng.dma_start(h[:], xt_d[b])
                hT[b] = h

            def load_adj(b, eng, c0, csz):
                if adj_sb[b] is None:
                    adj_sb[b] = adj_pool.tile([P, NT, N], F8, tag=f"adj{b}",
                                              name="a")
                srcb = adj_d[b].rearrange("(t p) n -> p t n", p=P)
                eng.dma_start(adj_sb[b][:, c0:c0 + csz, :],
                              srcb[:, c0:c0 + csz, :])

            # SP ring leads with (adj0-c0, x0.hi) then weights; ACT's
            # pre-compute window takes x0.lo/hT0 + b1's adjacency; later
            # batches split across both rings.  adj is fp8 (exact for the
            # 0/1 entries), so the whole stream is 4 MiB.
            def load_xhi(b, eng):
                hi = hnm_pool.tile([P, NT, C], F16, tag=f"hnm{b}", name="hi")
                eng.dma_start(hi[:], xh_d[b].rearrange("(t p) c -> p t c", p=P))
                h_nm[b] = hi

            def load_xlo(b, eng):
                lo = xlo_pool.tile([P, NT, C], F16, tag=f"xlo{b}", name="lo")
                eng.dma_start(lo[:], xl_d[b].rearrange("(t p) c -> p t c", p=P))
                x_lo[b] = lo

            load_xhi(0, nc.sync)
            load_adj(0, nc.sync, 2, 2)
            wch = consts.tile([P, L * 3 * C], F16)
            nc.sync.dma_start(wch[:], wch_d.rearrange("c l g d -> c (l g d)"))
            load_adj(0, nc.sync, 4, 2)
            wcl = consts.tile([P, L * 3 * C], F16)
            nc.sync.dma_start(wcl[:], wcl_d.rearrange("c l g d -> c (l g d)"))
            load_adj(0, nc.sync, 6, 2)
            whh = consts.tile([P, L * 3 * C], F16)
            nc.sync.dma_start(whh[:], whh_d.rearrange("c l g d -> c (l g d)"))
            bias = consts.tile([P, L * 4], F32)
            nc.sync.dma_start(bias[:], bias_d.rearrange("c l k -> c (l k)"))
            load_x(1, nc.sync)
            load_ht(1, nc.sync)

            # ACT ring: act-table warms can't lead (bias not loaded yet), so
            # x0's remaining planes + b1/b2 adjacency fill its window
            load_adj(0, nc.scalar, 0, 2)
            load_xlo(0, nc.scalar)
            load_ht(0, nc.scalar)
            load_adj(1, nc.scalar, 0, 4)
            load_adj(1, nc.scalar, 4, 4)
            # warm both activation-function tables inside the load window so
            # the implicit LoadActFuncSet pair never blocks the compute stream
            warm = consts.tile([P, 1], F32)
            nc.scalar.activation(warm[:], bias[:, 0:1], AF.Tanh)
            nc.scalar.activation(warm[:], bias[:, 0:1], AF.Sigmoid)
            load_x(2, nc.scalar)
            load_ht(2, nc.scalar)
            load_adj(2, nc.scalar, 0, 4)
            load_adj(2, nc.sync, 4, 4)
            load_x(3, nc.sync)
            load_ht(3, nc.sync)
            load_adj(3, nc.scalar, 0, 4)
            load_adj(3, nc.sync, 4, 4)

            # ---- layers ------------------------------------------------------
            for i in range(L):
                last_layer = i == L - 1
                s_split = i == 0        # layer 0: s-operand + s-cast hi/lo
                wc_split = WC_SPLIT[i]
                for b in range(BPC):
                    # sT = (A^T h)^T in psum (fp16 x fp8 matmul, fp32 accum)
                    planes = (h_nm[b], x_lo[b]) if s_split else (h_nm[b],)
                    s = sT_pool.tile([P, N], F16, tag="s")
                    if s_split:
                        s_lo = sT_pool.tile([P, N], F16, tag="slo")
                    for half in range(2):
                        hs = slice(half * HALF, (half + 1) * HALF)
                        ps = ps_s.tile([P, HALF], F32, tag="ps_s")
                        for pi, plane in enumerate(planes):
                            for j in range(NT):
                                nc.tensor.matmul(
                                    ps[:],
                                    lhsT=plane[:, j, :],
                                    rhs=adj_sb[b][:, j, hs],
                                    start=(pi == 0 and j == 0),
                                    stop=(pi == len(planes) - 1 and j == NT - 1),
                                )
                        nc.vector.tensor_copy(s[:, hs], ps[:])
                        if s_split:
                            nc.vector.tensor_sub(s_lo[:, hs], ps[:], s[:, hs])

                    new_h = hT_pool.tile([P, N], F16, tag=f"hT{b}")
                    tail = last_layer and b == BPC - 1
                    nq = 2 if tail else 1
                    qw = HALF // nq

                    # staged two-half emission: each engine queue gets both
                    # halves' independent ops back-to-back, so a chain wait
                    # on one half never head-blocks the other
                    rr, zz, uu = {}, {}, {}
                    for nh in range(2):
                        sl = slice(nh * HALF, (nh + 1) * HALF)
                        pr = ps_g.tile([P, HALF], F32, tag="psg")
                        phn = ps_g.tile([P, HALF], F32, tag="psg")
                        pxn = ps_g.tile([P, HALF], F32, tag="psg")
                        pz = ps_g.tile([P, HALF], F32, tag="psg")

                        def gate(pg, g, with_h):
                            mms = [(wslice(wch, i, g), s[:, sl])]
                            if s_split:
                                mms.append((wslice(wch, i, g), s_lo[:, sl]))
                            if wc_split:
                                mms.append((wslice(wcl, i, g), s[:, sl]))
                            if with_h:
                                mms.append((wslice(whh, i, g), hT[b][:, sl]))
                            for k, (lh, rh) in enumerate(mms):
                                nc.tensor.matmul(pg[:], lhsT=lh, rhs=rh,
                                                 start=(k == 0),
                                                 stop=(k == len(mms) - 1))

                        # emission order follows the consumer chain:
                        # r needs pr; t needs phn+r; u needs pxn+t; zc frees pz
                        gate(pr, 0, True)
                        nc.tensor.matmul(phn[:], lhsT=wslice(whh, i, 2),
                                         rhs=hT[b][:, sl], start=True, stop=True)
                        gate(pxn, 2, False)
                        gate(pz, 1, True)

                        r = ew_pool.tile([P, HALF], F32, tag="ew")
                        nc.scalar.activation(r[:], pr[:], AF.Sigmoid,
                                             bias=bslice(i, 0))
                        zc = ew_pool.tile([P, HALF], F32, tag="ew")
                        nc.scalar.activation(zc[:], pz[:], AF.Sigmoid,
                                             bias=bslice(i, 1), scale=-1.0)
                        t = ew_pool.tile([P, HALF], F32, tag="ew")
                        nc.vector.scalar_tensor_tensor(t[:], phn[:], bslice(i, 3),
                                                       r[:], op0=ALU.add,
                                                       op1=ALU.mult)
                        u = ew_pool.tile([P, HALF], F32, tag="ew")
                        nc.vector.scalar_tensor_tensor(u[:], pxn[:], bslice(i, 2),
                                                       t[:], op0=ALU.add,
                                                       op1=ALU.add)
                        rr[nh], zz[nh], uu[nh] = r, zc, u
                    if not last_layer:
                        nhi = hnm_pool.tile([P, NT, C], F16, tag=f"hnm{b}")
                    for nh in range(2):
                        for q in range(nq):
                            sl = slice(nh * HALF + q * qw,
                                       nh * HALF + (q + 1) * qw)
                            qs = slice(q * qw, (q + 1) * qw)
                            nt = ew_pool.tile([P, qw], F32, tag="ew")
                            nc.scalar.activation(nt[:], uu[nh][:, qs], AF.Tanh)
                            # the drain chain alternates Pool/DVE on the tail
                            # batch so it pipelines two-wide
                            veng = nc.vector if (tail and q == 1) else nc.gpsimd
                            d = ew_pool.tile([P, qw], F32, tag="ew")
                            veng.tensor_sub(d[:], nt[:], hT[b][:, sl])
                            e = ew_pool.tile([P, qw], F16, tag="ewh")
                            veng.tensor_mul(e[:], zz[nh][:, qs], d[:])
                            veng.tensor_add(new_h[:, sl], hT[b][:, sl], e[:])
                            if last_layer:
                                nc.sync.dma_start(out=y_d[b][:, sl],
                                                  in_=new_h[:, sl])
                        if not last_layer:
                            # stream the handoff per half: transpose nodes
                            # [nh*512, nh*512+512) so the next layer's
                            # s-matmuls start before the other half finishes
                            hsl = slice(nh * HALF, (nh + 1) * HALF)
                            tsl = slice(nh * (NT // 2), (nh + 1) * (NT // 2))
                            nc.sync.dma_start(out=nhi[:, tsl, :],
                                              in_=new_h[:, hsl], transpose=True)

                    hT[b] = new_h
                    if not last_layer:
                        h_nm[b] = nhi

    nc.finalize()
    return nc


def _prep_weights(weight, w_ih, w_hh, b_ih, b_hh):
    weight = np.asarray(weight, np.float32)
    w_ih = np.asarray(w_ih, np.float32)
    w_hh = np.asarray(w_hh, np.float32)
    b_ih = np.asarray(b_ih, np.float32)
    b_hh = np.asarray(b_hh, np.float32)

    # fused input-gate weight: gx = s @ (W @ w_ih^T), hi/lo fp16 split,
    # as [C, L, 3, C]
    wc = np.einsum("lcd,lgd->lcg", weight, w_ih)          # [L, C, 3C]
    wch = wc.astype(np.float16)
    wcl = (wc - wch.astype(np.float32)).astype(np.float16)
    whh_t = np.transpose(w_hh, (0, 2, 1)).astype(np.float16)  # [L, C, 3C]

    def to_clgd(a):  # [L, C, 3C] -> [C, L, 3, C]
        return np.ascontiguousarray(
            np.transpose(a.reshape(L, C, 3, C), (1, 0, 2, 3)))

    bias = np.empty((C, L, 4), np.float32)
    bias[:, :, 0] = (b_ih[:, 0:C] + b_hh[:, 0:C]).T
    bias[:, :, 1] = -(b_ih[:, C:2 * C] + b_hh[:, C:2 * C]).T
    bias[:, :, 2] = b_ih[:, 2 * C:3 * C].T
    bias[:, :, 3] = b_hh[:, 2 * C:3 * C].T

    return to_clgd(wch), to_clgd(wcl), to_clgd(whh_t), bias


def kernel(x, adj, mask, weight, w_ih, w_hh, b_ih, b_hh, _run_kwargs=None):
    xf = np.asarray(x, np.float32)                              # [B, N, C]
    x16h = xf.astype(np.float16)                                # hi plane
    x16l = (xf - x16h.astype(np.float32)).astype(np.float16)    # lo plane
    xt16 = np.ascontiguousarray(x16h.transpose(0, 2, 1))        # [B, C, N]
    # binary adjacency: fp8 is exact, quarters the HBM traffic on device
    adj8 = np.asarray(adj, np.float32).astype(mybir.dt.np(mybir.dt.float8e4))
    mask = np.asarray(mask, np.float32)
    wch, wcl, whh, bias = _prep_weights(weight, w_ih, w_hh, b_ih, b_hh)

    if "nc" not in _PROGRAM_CACHE:
        _PROGRAM_CACHE["nc"] = _build_program()
    nc = _PROGRAM_CACHE["nc"]

    in_maps = []
    for c in range(NCORES):
        sl = slice(c * BPC, (c + 1) * BPC)
        in_maps.append({
            "xh": np.ascontiguousarray(x16h[sl]),
            "xl": np.ascontiguousarray(x16l[sl]),
            "xt": np.ascontiguousarray(xt16[sl]),
            "adj": np.ascontiguousarray(adj8[sl]),
            "wch": wch, "wcl": wcl, "whh": whh, "bias": bias,
        })

    res = run_bass_kernel_spmd(nc, in_maps, list(range(NCORES)),
                               **(_run_kwargs or {}))
    # y is stored feature-major fp16 [BPC, C, N]; widen + transpose on host
    y = np.concatenate([r["y"] for r in res.results], axis=0)
    y = np.ascontiguousarray(y.transpose(0, 2, 1)).astype(np.float32)
    y = y * mask[:, :, None]
    if _run_kwargs:
        kernel.last_results = res
    return y
